# revision 31
# baseline (speedup 1.0000x reference)
"""Trainium2 Bass kernel for nn_Ag3SRModel (GNN message passing, 4096 atoms).

reference math:
  d_ij pairwise distances, mask = (d>0) & (d<5)
  rbf_k(d) = exp(-(d - k/3)^2 * 4.5), k=0..15
  features[i,k] = sum_j mask * rbf_k(d_ij)
  e = silu(features @ W1 + b1) @ W2 + b2 ; out = sum(e)

Device strategy (8 NeuronCores, SPMD, row-block over atoms i):
  - per core: 4 i-blocks of 128 atoms (partitions) x all 4096 j (free dim).
  - d^2 via augmented matmul (contraction K=5):
      lhsT = [-2X^T; 1; r][:, i-block]   rhs = [X^T; r; 1][:, j-half]
    into PSUM [128, 2048] halves (4 banks each, 2 bufs = 8 banks).
  - cutoff fold on d^2 per half (DVE, also clamps fp32-negative d^2 to 0):
      d'^2 = max(d^2, 144*(d^2>=25)) ; then d' = sqrt(d'^2) on ACT at 4096.
    masked pairs land at d' in [12, 26] where every rbf_k underflows to
    exactly 0 and exp(3 d') <= exp(78) stays finite.
  - unnormalized rbf chain anchored at k=8, pre-scaled by e^-SHIFT, all
    chain tensors bf16 (DVE tensor_tensor runs 2x in bf16):
      v_8 = exp(-4.5 (d'-8/3)^2 - SHIFT)        (Square+Exp on ACT)
      v_{k+1} = v_k * t,    t    = exp( 3 d')
      v_{k-1} = v_k * tinv, tinv = exp(-3 d')
    v_k = rbf_k / Q_k, Q_k = exp((64-k^2)/2 + SHIFT).
  - j-reductions of v_k split between DVE tensor_reduce and ACT
    Identity+accum_out to balance engine load; k=8 rides the anchor Exp.
  - source order is software-pipelined: tile ib+1's matmul/fold/ACT-core is
    emitted BEFORE tile ib's chains+reductions so ACT never stalls behind
    the DVE chain of the previous tile.
  - device outputs raw feature sums [512, 16]; host scales by Q_k, subtracts
    the diagonal rbf_k(0), runs the tiny MLP, sums energies.
"""

import math
import sys

sys.path.insert(0, "/opt/trn_rl_repo")

import numpy as np

import concourse.bass as bass
import concourse.tile as tile
from concourse import bacc, mybir
from concourse.bass_utils import run_bass_kernel_spmd

N = 4096
NCORES = 8
SLAB = N // NCORES          # 512 atoms i per core
P = 128                     # partitions
NIB = SLAB // P             # 4 i-blocks per core
JF = N                      # all j in one logical tile
HALF = 2048                 # psum half-tile
NRBF = 16
CUTOFF = 5.0
INV2W2 = 4.5                # 1/(2 w^2), w = 1/3
SQ = math.sqrt(INV2W2)
KA = 8                      # anchor k
CA = KA / 3.0
SHIFT = 48 * math.log(2.0)
F32 = mybir.dt.float32
BF16 = mybir.dt.bfloat16

# which k's reduce on DVE tensor_reduce (rest: ACT Identity+accum);
# first/last i-blocks lean DVE because ACT starts late / trails at the end
DVE_KS = (0, 4, 10, 14)
DVE_KS_FIRST = (0, 2, 4, 6, 10, 12, 14)
DVE_KS_LAST = (0, 2, 4, 6, 10, 12, 14)

_CACHE = {}


def _build():
    nc = bacc.Bacc("TRN2", target_bir_lowering=False, debug=False,
                   num_devices=NCORES)

    ab_d = nc.dram_tensor("AB", [5, N + SLAB], F32, kind="ExternalInput").ap()
    feats_d = nc.dram_tensor("feats", [SLAB, NRBF], F32, kind="ExternalOutput").ap()

    with tile.TileContext(nc) as tc:
        with (
            tc.tile_pool(name="singles", bufs=1) as singles,
            tc.tile_pool(name="w1", bufs=1) as w1p,
            tc.tile_pool(name="w2", bufs=2) as w2p,
            tc.tile_pool(name="chainp", bufs=6) as chainp,
            tc.tile_pool(name="facc", bufs=4) as faccp,
            tc.tile_pool(name="psum_d2", bufs=2, space="PSUM") as psum_d2,
        ):
            ab_sb = singles.tile([5, N + SLAB], F32)
            nc.sync.dma_start(out=ab_sb, in_=ab_d)
            bias8_sb = singles.tile([P, 1], F32)
            nc.vector.memset(bias8_sb, -SQ * CA)
            biasS_sb = singles.tile([P, 1], F32)
            nc.vector.memset(biasS_sb, -SHIFT)

            dummy = singles.tile([P, 1], BF16)

            def head_fold(ib):
                """matmul + cutoff fold (PE + DVE) for i-block ib."""
                lhsT = ab_sb[:, N + ib * P: N + (ib + 1) * P]
                d2c = w2p.tile([P, JF], F32, tag="d2c")
                for half in range(JF // HALF):
                    d2_ps = psum_d2.tile([P, HALF], F32, tag="d2")
                    for h in range(HALF // 512):
                        c0 = half * HALF + h * 512
                        nc.tensor.matmul(
                            d2_ps[:, h * 512:(h + 1) * 512],
                            lhsT, ab_sb[:, c0:c0 + 512],
                            start=True, stop=True,
                        )
                    m144 = w1p.tile([P, HALF], F32, tag="m144")
                    nc.vector.tensor_scalar(m144, d2_ps, CUTOFF * CUTOFF,
                                            144.0, mybir.AluOpType.is_ge,
                                            mybir.AluOpType.mult)
                    nc.vector.tensor_tensor(
                        d2c[:, half * HALF:(half + 1) * HALF], d2_ps, m144,
                        mybir.AluOpType.max)
                return d2c

            def make_core(ib, d2c):
                """Allocate ACT-core tiles; return emit-closures + tiles."""
                dp = w1p.tile([P, JF], F32, tag="dp")
                s8 = w1p.tile([P, JF], F32, tag="s8")
                t = w2p.tile([P, JF], BF16, tag="t")
                tinv = w2p.tile([P, JF], BF16, tag="tinv")
                v8 = w2p.tile([P, JF], BF16, tag="v8")
                fraw = faccp.tile([P, NRBF], F32, tag="fraw")
                A = mybir.ActivationFunctionType
                ops = [
                    lambda: nc.scalar.activation(dp, d2c, A.Sqrt),
                    lambda: nc.scalar.activation(s8, dp, A.Square,
                                                 bias=bias8_sb, scale=SQ),
                    lambda: nc.scalar.activation(t, dp, A.Exp, scale=3.0),
                    lambda: nc.scalar.activation(tinv, dp, A.Exp, scale=-3.0),
                    lambda: nc.scalar.activation(
                        v8, s8, A.Exp, scale=-1.0, bias=biasS_sb,
                        accum_out=fraw[:, KA:KA + 1]),
                ]
                return ops, (t, tinv, v8, fraw)

            def tail(ib, tiles, next_ops, last=False):
                """chains + reductions for i-block ib, interleaving the next
                i-block's ACT-core ops so ACT serves reductions promptly."""
                t, tinv, v8, fraw = tiles
                dve_ks = (DVE_KS_LAST if last
                          else DVE_KS_FIRST if ib == 0 else DVE_KS)
                inject_at = {1: 0, 4: 1, 7: 2, 9: 3, 11: 4}
                step = 0

                def emit_reduce(k, v):
                    nonlocal step
                    col = fraw[:, k:k + 1]
                    if k in dve_ks:
                        nc.vector.tensor_reduce(col, v,
                                                mybir.AxisListType.X,
                                                mybir.AluOpType.add)
                    else:
                        nc.scalar.activation(
                            dummy.broadcast_to((P, JF)), v,
                            mybir.ActivationFunctionType.Identity,
                            accum_out=col)
                    if next_ops is not None and step in inject_at:
                        next_ops[inject_at[step]]()
                    step += 1

                v = v8
                for k in range(KA + 1, NRBF):       # chain up
                    vn = chainp.tile([P, JF], BF16, tag="vchain")
                    nc.vector.tensor_tensor(vn, v, t, mybir.AluOpType.mult)
                    emit_reduce(k, vn)
                    v = vn
                v = v8
                for k in range(KA - 1, -1, -1):     # chain down
                    vn = chainp.tile([P, JF], BF16, tag="vchain")
                    nc.vector.tensor_tensor(vn, v, tinv,
                                            mybir.AluOpType.mult)
                    emit_reduce(k, vn)
                    v = vn
                nc.sync.dma_start(out=feats_d[ib * P:(ib + 1) * P, :],
                                  in_=fraw)

            # software pipeline: fold(ib+1) precedes chains(ib) on DVE;
            # core(ib+1) ACT ops are interleaved among reductions(ib)
            ops0, tiles0 = make_core(0, head_fold(0))
            for op in ops0:
                op()
            prev = tiles0
            for ib in range(1, NIB):
                opsN, tilesN = make_core(ib, head_fold(ib))
                tail(ib - 1, prev, opsN)
                prev = tilesN
            tail(NIB - 1, prev, None, last=True)

    nc.compile()
    return nc


def kernel(positions, W1, b1, W2, b2):
    positions = np.asarray(positions, dtype=np.float32)
    W1 = np.asarray(W1, dtype=np.float32)
    b1 = np.asarray(b1, dtype=np.float32)
    W2 = np.asarray(W2, dtype=np.float32)
    b2 = np.asarray(b2, dtype=np.float32)

    if "nc" not in _CACHE:
        _CACHE["nc"] = _build()
    nc = _CACHE["nc"]

    r = (positions.astype(np.float64) ** 2).sum(axis=1)
    xt = positions.T.astype(np.float64)                    # [3, N]
    A = np.concatenate([xt, r[None, :], np.ones((1, N))])              # [5, N]
    B = np.concatenate([-2.0 * xt, np.ones((1, N)), r[None, :]])       # [5, N]

    in_maps = [
        {"AB": np.concatenate(
            [A, B[:, c * SLAB:(c + 1) * SLAB]], axis=1).astype(np.float32)}
        for c in range(NCORES)
    ]
    res = run_bass_kernel_spmd(nc, in_maps, list(range(NCORES)))
    feats_raw = np.concatenate([res.results[c]["feats"] for c in range(NCORES)])

    ks = np.arange(NRBF, dtype=np.float64)
    Q = np.exp((KA * KA - ks * ks) / 2.0 + SHIFT)
    ek = np.exp(-0.5 * ks * ks)             # diagonal rbf_k(0)
    f = (feats_raw.astype(np.float64) * Q - ek).astype(np.float32)

    z = (f @ W1 + b1).astype(np.float64)
    h = z * 0.5 * (1.0 + np.tanh(0.5 * z))  # silu, overflow-safe
    e = h @ W2.reshape(-1, 1) + b2.reshape(1, -1)
    return np.float32(e.sum())


# revision 32
# speedup vs baseline: 1.0123x; 1.0123x over previous
"""Trainium2 Bass kernel for nn_Ag3SRModel (GNN message passing, 4096 atoms).

reference math:
  d_ij pairwise distances, mask = (d>0) & (d<5)
  rbf_k(d) = exp(-(d - k/3)^2 * 4.5), k=0..15
  features[i,k] = sum_j mask * rbf_k(d_ij)
  e = silu(features @ W1 + b1) @ W2 + b2 ; out = sum(e)

Device strategy (8 NeuronCores, SPMD, row-block over atoms i):
  - per core: 4 i-blocks of 128 atoms (partitions) x all 4096 j (free dim).
  - d^2 via augmented matmul (contraction K=5):
      lhsT = [-2X^T; 1; r][:, i-block]   rhs = [X^T; r; 1][:, j-half]
    into PSUM [128, 2048] halves (4 banks each, 2 bufs = 8 banks).
  - cutoff fold on d^2 per half (DVE, also clamps fp32-negative d^2 to 0):
      d'^2 = max(d^2, 144*(d^2>=25)) ; then d' = sqrt(d'^2) on ACT at 4096.
    masked pairs land at d' in [12, 26] where every rbf_k underflows to
    exactly 0 and exp(3 d') <= exp(78) stays finite.
  - unnormalized rbf chain anchored at k=8, pre-scaled by e^-SHIFT, all
    chain tensors bf16 (DVE tensor_tensor runs 2x in bf16):
      v_8 = exp(-4.5 (d'-8/3)^2 - SHIFT)        (Square+Exp on ACT)
      v_{k+1} = v_k * t,    t    = exp( 3 d')
      v_{k-1} = v_k * tinv, tinv = exp(-3 d')
    v_k = rbf_k / Q_k, Q_k = exp((64-k^2)/2 + SHIFT).
  - j-reductions of v_k split between DVE tensor_reduce and ACT
    Identity+accum_out to balance engine load; k=8 rides the anchor Exp.
  - source order is software-pipelined: tile ib+1's matmul/fold/ACT-core is
    emitted BEFORE tile ib's chains+reductions so ACT never stalls behind
    the DVE chain of the previous tile.
  - device outputs raw feature sums [512, 16]; host scales by Q_k, subtracts
    the diagonal rbf_k(0), runs the tiny MLP, sums energies.
"""

import math
import sys

sys.path.insert(0, "/opt/trn_rl_repo")

import numpy as np

import concourse.bass as bass
import concourse.tile as tile
from concourse import bacc, mybir
from concourse.bass_utils import run_bass_kernel_spmd

N = 4096
NCORES = 8
SLAB = N // NCORES          # 512 atoms i per core
P = 128                     # partitions
NIB = SLAB // P             # 4 i-blocks per core
JF = N                      # all j in one logical tile
HALF = 2048                 # psum half-tile
NRBF = 16
CUTOFF = 5.0
INV2W2 = 4.5                # 1/(2 w^2), w = 1/3
SQ = math.sqrt(INV2W2)
KA = 8                      # anchor k
CA = KA / 3.0
SHIFT = 48 * math.log(2.0)
F32 = mybir.dt.float32
BF16 = mybir.dt.bfloat16

# which k's reduce on DVE tensor_reduce (rest: ACT Identity+accum);
# first/last i-blocks lean DVE because ACT starts late / trails at the end
DVE_KS = (0, 4, 10, 14)
DVE_KS_FIRST = (0, 2, 4, 6, 10, 12, 14)
DVE_KS_LAST = (0, 2, 4, 6, 10, 12, 14)

_CACHE = {}


def _build():
    nc = bacc.Bacc("TRN2", target_bir_lowering=False, debug=False,
                   num_devices=NCORES)

    ab_d = nc.dram_tensor("AB", [5, N + SLAB], F32, kind="ExternalInput").ap()
    feats_d = nc.dram_tensor("feats", [SLAB, NRBF], F32, kind="ExternalOutput").ap()

    with tile.TileContext(nc) as tc:
        with (
            tc.tile_pool(name="singles", bufs=1) as singles,
            tc.tile_pool(name="w1", bufs=1) as w1p,
            tc.tile_pool(name="w2", bufs=2) as w2p,
            tc.tile_pool(name="chainp", bufs=8) as chainp,
            tc.tile_pool(name="facc", bufs=4) as faccp,
            tc.tile_pool(name="psum_d2", bufs=2, space="PSUM") as psum_d2,
        ):
            ab_sb = singles.tile([5, N + SLAB], F32)
            nc.sync.dma_start(out=ab_sb, in_=ab_d)
            bias8_sb = singles.tile([P, 1], F32)
            nc.vector.memset(bias8_sb, -SQ * CA)
            biasS_sb = singles.tile([P, 1], F32)
            nc.vector.memset(biasS_sb, -SHIFT)

            dummy = singles.tile([P, 1], BF16)

            def head_fold(ib):
                """matmul + cutoff fold (PE + DVE) for i-block ib."""
                lhsT = ab_sb[:, N + ib * P: N + (ib + 1) * P]
                d2c = w1p.tile([P, JF], F32, tag="d2c")
                for half in range(JF // HALF):
                    d2_ps = psum_d2.tile([P, HALF], F32, tag="d2")
                    for h in range(HALF // 512):
                        c0 = half * HALF + h * 512
                        nc.tensor.matmul(
                            d2_ps[:, h * 512:(h + 1) * 512],
                            lhsT, ab_sb[:, c0:c0 + 512],
                            start=True, stop=True,
                        )
                    m144 = w1p.tile([P, HALF], F32, tag="m144")
                    nc.vector.tensor_scalar(m144, d2_ps, CUTOFF * CUTOFF,
                                            144.0, mybir.AluOpType.is_ge,
                                            mybir.AluOpType.mult)
                    nc.vector.tensor_tensor(
                        d2c[:, half * HALF:(half + 1) * HALF], d2_ps, m144,
                        mybir.AluOpType.max)
                return d2c

            def make_core(ib, d2c):
                """Allocate ACT-core tiles; return emit-closures + tiles."""
                dp = w1p.tile([P, JF], F32, tag="dp")
                s8 = w1p.tile([P, JF], F32, tag="s8")
                t = w2p.tile([P, JF], BF16, tag="t")
                tinv = w2p.tile([P, JF], BF16, tag="tinv")
                v8 = w2p.tile([P, JF], BF16, tag="v8")
                fraw = faccp.tile([P, NRBF], F32, tag="fraw")
                A = mybir.ActivationFunctionType
                ops = [
                    lambda: nc.scalar.activation(dp, d2c, A.Sqrt),
                    lambda: nc.scalar.activation(s8, dp, A.Square,
                                                 bias=bias8_sb, scale=SQ),
                    lambda: nc.scalar.activation(t, dp, A.Exp, scale=3.0),
                    lambda: nc.scalar.activation(tinv, dp, A.Exp, scale=-3.0),
                    lambda: nc.scalar.activation(
                        v8, s8, A.Exp, scale=-1.0, bias=biasS_sb,
                        accum_out=fraw[:, KA:KA + 1]),
                ]
                return ops, (t, tinv, v8, fraw)

            def tail(ib, tiles, next_ops, last=False):
                """chains + reductions for i-block ib, interleaving the next
                i-block's ACT-core ops so ACT serves reductions promptly."""
                t, tinv, v8, fraw = tiles
                dve_ks = (DVE_KS_LAST if last
                          else DVE_KS_FIRST if ib == 0 else DVE_KS)
                inject_at = {1: 0, 4: 1, 7: 2, 9: 3, 11: 4}
                step = 0

                def emit_reduce(k, v):
                    nonlocal step
                    col = fraw[:, k:k + 1]
                    if k in dve_ks:
                        nc.vector.tensor_reduce(col, v,
                                                mybir.AxisListType.X,
                                                mybir.AluOpType.add)
                    else:
                        nc.scalar.activation(
                            dummy.broadcast_to((P, JF)), v,
                            mybir.ActivationFunctionType.Identity,
                            accum_out=col)
                    if next_ops is not None and step in inject_at:
                        next_ops[inject_at[step]]()
                    step += 1

                v = v8
                for k in range(KA + 1, NRBF):       # chain up
                    vn = chainp.tile([P, JF], BF16, tag="vchain")
                    nc.vector.tensor_tensor(vn, v, t, mybir.AluOpType.mult)
                    emit_reduce(k, vn)
                    v = vn
                v = v8
                for k in range(KA - 1, -1, -1):     # chain down
                    vn = chainp.tile([P, JF], BF16, tag="vchain")
                    nc.vector.tensor_tensor(vn, v, tinv,
                                            mybir.AluOpType.mult)
                    emit_reduce(k, vn)
                    v = vn
                nc.sync.dma_start(out=feats_d[ib * P:(ib + 1) * P, :],
                                  in_=fraw)

            # software pipeline: fold(ib+1) precedes chains(ib) on DVE;
            # core(ib+1) ACT ops are interleaved among reductions(ib)
            ops0, tiles0 = make_core(0, head_fold(0))
            for op in ops0:
                op()
            prev = tiles0
            for ib in range(1, NIB):
                opsN, tilesN = make_core(ib, head_fold(ib))
                tail(ib - 1, prev, opsN)
                prev = tilesN
            tail(NIB - 1, prev, None, last=True)

    nc.compile()
    return nc


def kernel(positions, W1, b1, W2, b2):
    positions = np.asarray(positions, dtype=np.float32)
    W1 = np.asarray(W1, dtype=np.float32)
    b1 = np.asarray(b1, dtype=np.float32)
    W2 = np.asarray(W2, dtype=np.float32)
    b2 = np.asarray(b2, dtype=np.float32)

    if "nc" not in _CACHE:
        _CACHE["nc"] = _build()
    nc = _CACHE["nc"]

    r = (positions.astype(np.float64) ** 2).sum(axis=1)
    xt = positions.T.astype(np.float64)                    # [3, N]
    A = np.concatenate([xt, r[None, :], np.ones((1, N))])              # [5, N]
    B = np.concatenate([-2.0 * xt, np.ones((1, N)), r[None, :]])       # [5, N]

    in_maps = [
        {"AB": np.concatenate(
            [A, B[:, c * SLAB:(c + 1) * SLAB]], axis=1).astype(np.float32)}
        for c in range(NCORES)
    ]
    res = run_bass_kernel_spmd(nc, in_maps, list(range(NCORES)))
    feats_raw = np.concatenate([res.results[c]["feats"] for c in range(NCORES)])

    ks = np.arange(NRBF, dtype=np.float64)
    Q = np.exp((KA * KA - ks * ks) / 2.0 + SHIFT)
    ek = np.exp(-0.5 * ks * ks)             # diagonal rbf_k(0)
    f = (feats_raw.astype(np.float64) * Q - ek).astype(np.float32)

    z = (f @ W1 + b1).astype(np.float64)
    h = z * 0.5 * (1.0 + np.tanh(0.5 * z))  # silu, overflow-safe
    e = h @ W2.reshape(-1, 1) + b2.reshape(1, -1)
    return np.float32(e.sum())


# revision 33
# speedup vs baseline: 1.0718x; 1.0588x over previous
"""Trainium2 Bass kernel for nn_Ag3SRModel (GNN message passing, 4096 atoms).

reference math:
  d_ij pairwise distances, mask = (d>0) & (d<5)
  rbf_k(d) = exp(-(d - k/3)^2 * 4.5), k=0..15
  features[i,k] = sum_j mask * rbf_k(d_ij)
  e = silu(features @ W1 + b1) @ W2 + b2 ; out = sum(e)

Device strategy (8 NeuronCores, SPMD, row-block over atoms i):
  - per core: 4 i-blocks of 128 atoms (partitions) x all 4096 j (free dim).
  - d^2 via augmented matmul (contraction K=5):
      lhsT = [-2X^T; 1; r][:, i-block]   rhs = [X^T; r; 1][:, j-half]
    into PSUM [128, 2048] halves (4 banks each, 2 bufs = 8 banks).
  - cutoff fold on d^2 per half (DVE, also clamps fp32-negative d^2 to 0):
      d'^2 = max(d^2, 144*(d^2>=25)) ; then d' = sqrt(d'^2) on ACT at 4096.
    masked pairs land at d' in [12, 26] where every rbf_k underflows to
    exactly 0 and exp(3 d') <= exp(78) stays finite.
  - unnormalized rbf chain anchored at k=8, pre-scaled by e^-SHIFT, all
    chain tensors bf16 (DVE tensor_tensor runs 2x in bf16):
      v_8 = exp(-4.5 (d'-8/3)^2 - SHIFT)        (Square+Exp on ACT)
      v_{k+1} = v_k * t,    t    = exp( 3 d')
      v_{k-1} = v_k * tinv, tinv = exp(-3 d')
    v_k = rbf_k / Q_k, Q_k = exp((64-k^2)/2 + SHIFT).
  - j-reductions of v_k split between DVE tensor_reduce and ACT
    Identity+accum_out to balance engine load; k=8 rides the anchor Exp.
  - source order is software-pipelined: tile ib+1's matmul/fold/ACT-core is
    emitted BEFORE tile ib's chains+reductions so ACT never stalls behind
    the DVE chain of the previous tile.
  - device outputs raw feature sums [512, 16]; host scales by Q_k, subtracts
    the diagonal rbf_k(0), runs the tiny MLP, sums energies.
"""

import math
import sys

sys.path.insert(0, "/opt/trn_rl_repo")

import numpy as np

import concourse.bass as bass
import concourse.tile as tile
from concourse import bacc, mybir
from concourse.bass_utils import run_bass_kernel_spmd

N = 4096
NCORES = 8
SLAB = N // NCORES          # 512 atoms i per core
P = 128                     # partitions
NIB = SLAB // P             # 4 i-blocks per core
JF = N                      # all j in one logical tile
HALF = 2048                 # psum half-tile
NRBF = 16
CUTOFF = 5.0
INV2W2 = 4.5                # 1/(2 w^2), w = 1/3
SQ = math.sqrt(INV2W2)
KA = 8                      # anchor k
CA = KA / 3.0
SHIFT = 48 * math.log(2.0)
F32 = mybir.dt.float32
BF16 = mybir.dt.bfloat16

# which k's reduce on DVE tensor_reduce (rest: ACT Identity+accum);
# first/last i-blocks lean DVE because ACT starts late / trails at the end
DVE_KS = (0, 4, 10, 14)
DVE_KS_FIRST = (0, 4, 10, 14)
DVE_KS_LAST = (0, 4, 6, 10, 14)

_CACHE = {}


def _build():
    nc = bacc.Bacc("TRN2", target_bir_lowering=False, debug=False,
                   num_devices=NCORES)

    ab_d = nc.dram_tensor("AB", [5, N + SLAB], F32, kind="ExternalInput").ap()
    feats_d = nc.dram_tensor("feats", [SLAB, NRBF], F32, kind="ExternalOutput").ap()

    with tile.TileContext(nc) as tc:
        with (
            tc.tile_pool(name="singles", bufs=1) as singles,
            tc.tile_pool(name="w1", bufs=1) as w1p,
            tc.tile_pool(name="w2", bufs=2) as w2p,
            tc.tile_pool(name="chainp", bufs=8) as chainp,
            tc.tile_pool(name="facc", bufs=4) as faccp,
            tc.tile_pool(name="psum_d2", bufs=2, space="PSUM") as psum_d2,
        ):
            ab_sb = singles.tile([5, N + SLAB], F32)
            nc.sync.dma_start(out=ab_sb, in_=ab_d)
            bias8_sb = singles.tile([P, 1], F32)
            nc.vector.memset(bias8_sb, -SQ * CA)
            biasS_sb = singles.tile([P, 1], F32)
            nc.vector.memset(biasS_sb, -SHIFT)

            dummy = singles.tile([P, 1], BF16)

            def head_fold(ib):
                """matmul + cutoff fold (PE + DVE) for i-block ib."""
                lhsT = ab_sb[:, N + ib * P: N + (ib + 1) * P]
                d2c = w1p.tile([P, JF], F32, tag="d2c")
                for half in range(JF // HALF):
                    d2_ps = psum_d2.tile([P, HALF], F32, tag="d2")
                    for h in range(HALF // 512):
                        c0 = half * HALF + h * 512
                        nc.tensor.matmul(
                            d2_ps[:, h * 512:(h + 1) * 512],
                            lhsT, ab_sb[:, c0:c0 + 512],
                            start=True, stop=True,
                        )
                    m144 = w1p.tile([P, HALF], F32, tag="m144")
                    nc.vector.tensor_scalar(m144, d2_ps, CUTOFF * CUTOFF,
                                            144.0, mybir.AluOpType.is_ge,
                                            mybir.AluOpType.mult)
                    nc.vector.tensor_tensor(
                        d2c[:, half * HALF:(half + 1) * HALF], d2_ps, m144,
                        mybir.AluOpType.max)
                return d2c

            def make_core(ib, d2c):
                """Allocate ACT-core tiles; return emit-closures + tiles."""
                dp = w1p.tile([P, JF], F32, tag="dp")
                s8 = w1p.tile([P, JF], F32, tag="s8")
                t = w2p.tile([P, JF], BF16, tag="t")
                tinv = w2p.tile([P, JF], BF16, tag="tinv")
                v8 = w2p.tile([P, JF], BF16, tag="v8")
                fraw = faccp.tile([P, NRBF], F32, tag="fraw")
                A = mybir.ActivationFunctionType
                ops = [
                    lambda: nc.scalar.activation(dp, d2c, A.Sqrt),
                    lambda: nc.scalar.activation(s8, dp, A.Square,
                                                 bias=bias8_sb, scale=SQ),
                    lambda: nc.scalar.activation(t, dp, A.Exp, scale=3.0),
                    lambda: nc.scalar.activation(tinv, dp, A.Exp, scale=-3.0),
                    lambda: nc.scalar.activation(
                        v8, s8, A.Exp, scale=-1.0, bias=biasS_sb,
                        accum_out=fraw[:, KA:KA + 1]),
                ]
                return ops, (t, tinv, v8, fraw)

            def tail(ib, tiles, next_ops, last=False):
                """chains + reductions for i-block ib, interleaving the next
                i-block's ACT-core ops so ACT serves reductions promptly."""
                t, tinv, v8, fraw = tiles
                dve_ks = (DVE_KS_LAST if last
                          else DVE_KS_FIRST if ib == 0 else DVE_KS)
                inject_at = {1: 0, 4: 1, 7: 2, 9: 3, 11: 4}
                step = 0

                def emit_reduce(k, v):
                    nonlocal step
                    col = fraw[:, k:k + 1]
                    if k in dve_ks:
                        nc.vector.tensor_reduce(col, v,
                                                mybir.AxisListType.X,
                                                mybir.AluOpType.add)
                    else:
                        nc.scalar.activation(
                            dummy.broadcast_to((P, JF)), v,
                            mybir.ActivationFunctionType.Identity,
                            accum_out=col)
                    if next_ops is not None and step in inject_at:
                        next_ops[inject_at[step]]()
                    step += 1

                v = v8
                for k in range(KA + 1, NRBF):       # chain up
                    vn = chainp.tile([P, JF], BF16, tag="vchain")
                    nc.vector.tensor_tensor(vn, v, t, mybir.AluOpType.mult)
                    emit_reduce(k, vn)
                    v = vn
                v = v8
                for k in range(KA - 1, -1, -1):     # chain down
                    vn = chainp.tile([P, JF], BF16, tag="vchain")
                    nc.vector.tensor_tensor(vn, v, tinv,
                                            mybir.AluOpType.mult)
                    emit_reduce(k, vn)
                    v = vn
                nc.sync.dma_start(out=feats_d[ib * P:(ib + 1) * P, :],
                                  in_=fraw)

            # software pipeline: fold(ib+1) precedes chains(ib) on DVE;
            # core(ib+1) ACT ops are interleaved among reductions(ib)
            ops0, tiles0 = make_core(0, head_fold(0))
            for op in ops0:
                op()
            prev = tiles0
            for ib in range(1, NIB):
                opsN, tilesN = make_core(ib, head_fold(ib))
                tail(ib - 1, prev, opsN)
                prev = tilesN
            tail(NIB - 1, prev, None, last=True)

    nc.compile()
    return nc


def kernel(positions, W1, b1, W2, b2):
    positions = np.asarray(positions, dtype=np.float32)
    W1 = np.asarray(W1, dtype=np.float32)
    b1 = np.asarray(b1, dtype=np.float32)
    W2 = np.asarray(W2, dtype=np.float32)
    b2 = np.asarray(b2, dtype=np.float32)

    if "nc" not in _CACHE:
        _CACHE["nc"] = _build()
    nc = _CACHE["nc"]

    r = (positions.astype(np.float64) ** 2).sum(axis=1)
    xt = positions.T.astype(np.float64)                    # [3, N]
    A = np.concatenate([xt, r[None, :], np.ones((1, N))])              # [5, N]
    B = np.concatenate([-2.0 * xt, np.ones((1, N)), r[None, :]])       # [5, N]

    in_maps = [
        {"AB": np.concatenate(
            [A, B[:, c * SLAB:(c + 1) * SLAB]], axis=1).astype(np.float32)}
        for c in range(NCORES)
    ]
    res = run_bass_kernel_spmd(nc, in_maps, list(range(NCORES)))
    feats_raw = np.concatenate([res.results[c]["feats"] for c in range(NCORES)])

    ks = np.arange(NRBF, dtype=np.float64)
    Q = np.exp((KA * KA - ks * ks) / 2.0 + SHIFT)
    ek = np.exp(-0.5 * ks * ks)             # diagonal rbf_k(0)
    f = (feats_raw.astype(np.float64) * Q - ek).astype(np.float32)

    z = (f @ W1 + b1).astype(np.float64)
    h = z * 0.5 * (1.0 + np.tanh(0.5 * z))  # silu, overflow-safe
    e = h @ W2.reshape(-1, 1) + b2.reshape(1, -1)
    return np.float32(e.sum())
